# revision 11
# baseline (speedup 1.0000x reference)
"""CosineEncoderBlock on 8 TRN2 NeuronCores.

Strategy
--------
Data-parallel over the 16384 query rows (2048 per core); prototypes and
weights replicated.  The cosine attention has no softmax, so it is linear
attention:  (q_hat @ k_hat.T) @ v  ==  q_hat @ (k_hat.T @ v)  per head.
Each per-head 64x64 matrix M_h = k_hat_h.T @ v_h is folded together with
the output projection into one 1024x1024 matrix
    W_tilde = vstack_h(M_h @ wo[h*64:(h+1)*64, :]),
collapsing attention+wo into a single dense matmul on q_hat.

Activations live feature-major (features on SBUF partitions, rows on the
free axis).  Row statistics are ones-vector matmuls; per-row broadcast
back across partitions is a rank-1 matmul.  LN elementwise weight/bias
are folded into the following projection host-side.

v2 over the original baseline:
 - every weight is SBUF-resident (w1/w2 stored fp8e4m3 scaled x16, with
   the descale folded into the gelu activation scale / FFN2 epilogue),
   so nothing streams from HBM inside the block loop;
 - prototype k/v projections are computed row-major directly
   (out = ph.T @ w), removing all 128 PE transposes;
 - reciprocal_approx_fast / affine_then_add / Square(bias=) fusions cut
   the DVE+ACT critical path;
 - deeper PSUM rotation (4 matmul banks) + double-buffered block tiles
   keep the PE warm (HAM K=8/8) across block boundaries.
"""

import os

import numpy as np
from contextlib import ExitStack

DIM = 1024
HEADS = 16
DH = 64
INNER = HEADS * DH
MLP = 4096
NQ = 16384
NW = 1024
N_CORES = 8
RPC = NQ // N_CORES          # rows per core = 2048
R = 256                      # rows per block
NBLK = RPC // R              # 8 blocks per core
KC = DIM // 128              # 8 feature chunks
MC1 = MLP // 128             # 32 mlp chunks
LN_EPS = 1e-5
S1 = 16.0                    # host-side scale on w1 (fp8 dynamic range)
S2 = 16.0                    # host-side scale on w2

_BUILD_CACHE = {}


def _build_nc():
    import concourse.bacc as bacc
    import concourse.mybir as mybir
    import concourse.tile as tile

    f32 = mybir.dt.float32
    f32r = mybir.dt.float32r
    bf16 = mybir.dt.bfloat16
    fp8 = mybir.dt.float8e4
    ADD = mybir.AluOpType.add
    SUB = mybir.AluOpType.subtract
    MUL = mybir.AluOpType.mult
    AF = mybir.ActivationFunctionType

    nc = bacc.Bacc("TRN2", target_bir_lowering=False, debug=False,
                   num_devices=N_CORES)

    # ---- DRAM I/O ----
    d_qT = nc.dram_tensor("qT", (DIM, RPC), f32, kind="ExternalInput").ap()
    d_pT = nc.dram_tensor("pT", (DIM, NW), f32, kind="ExternalInput").ap()
    d_wq = nc.dram_tensor("wq_f", (DIM, DIM), bf16, kind="ExternalInput").ap()
    d_wk = nc.dram_tensor("wk_f", (DIM, DIM), bf16, kind="ExternalInput").ap()
    d_wv = nc.dram_tensor("wv_f", (DIM, DIM), bf16, kind="ExternalInput").ap()
    d_wo = nc.dram_tensor("wo_b", (INNER, DIM), bf16, kind="ExternalInput").ap()
    d_w1 = nc.dram_tensor("w1_8", (DIM, MLP), fp8, kind="ExternalInput").ap()
    d_w2 = nc.dram_tensor("w2_8", (MLP, DIM), fp8, kind="ExternalInput").ap()
    d_cq = nc.dram_tensor("cq_c", (128, KC), f32, kind="ExternalInput").ap()
    d_ck = nc.dram_tensor("ck_r", (1, DIM), f32, kind="ExternalInput").ap()
    d_cv = nc.dram_tensor("cv_r", (1, DIM), f32, kind="ExternalInput").ap()
    d_bo = nc.dram_tensor("bo_c", (128, KC), f32, kind="ExternalInput").ap()
    d_b2 = nc.dram_tensor("b2_c", (128, KC), f32, kind="ExternalInput").ap()
    d_b1 = nc.dram_tensor("b1_c", (128, MC1), f32, kind="ExternalInput").ap()
    d_ohp = nc.dram_tensor("ohp", (128, 2), bf16, kind="ExternalInput").ap()
    d_on2 = nc.dram_tensor("on2", (2, 128), f32, kind="ExternalInput").ap()
    d_out = nc.dram_tensor("yT", (DIM, RPC), f32, kind="ExternalOutput").ap()

    DBG = bool(os.environ.get("KERNEL_DEBUG"))
    if DBG:
        d_adbg = nc.dram_tensor("a_dbg", (128, KC, RPC), f32,
                                kind="ExternalOutput").ap()
        d_wtdbg = nc.dram_tensor("wt_dbg", (128, KC, DIM), bf16,
                                 kind="ExternalOutput").ap()
        d_khdbg = nc.dram_tensor("kh_dbg", (128, KC, INNER), bf16,
                                 kind="ExternalOutput").ap()
        d_vdbg = nc.dram_tensor("v_dbg", (128, KC, INNER), bf16,
                                kind="ExternalOutput").ap()
        d_qhdbg = nc.dram_tensor("qh_dbg", (128, KC, RPC), bf16,
                                 kind="ExternalOutput").ap()

    qT3 = d_qT.rearrange("(c p) r -> p c r", p=128)
    pT3 = d_pT.rearrange("(c p) r -> p c r", p=128)
    out3 = d_out.rearrange("(c p) r -> p c r", p=128)

    with ExitStack() as ctx:
        tc = ctx.enter_context(tile.TileContext(nc))
        ctx.enter_context(nc.allow_low_precision(
            reason="bf16 intermediates + fp8 ffn weights, tol 2e-2"))
        sg = ctx.enter_context(tc.tile_pool(name="singles", bufs=1))

        # --- resident weights / constants ---
        wqS = sg.tile([128, KC, DIM], bf16)
        nc.sync.dma_start(out=wqS, in_=d_wq.rearrange("(k p) m -> p k m", p=128))
        wtS = sg.tile([128, KC, DIM], bf16)   # W_tilde, written on device
        w1S = sg.tile([128, KC, MLP], fp8)
        nc.sync.dma_start(out=w1S, in_=d_w1.rearrange("(k p) m -> p k m", p=128))
        w2S = sg.tile([128, MC1, DIM], fp8)
        nc.sync.dma_start(out=w2S, in_=d_w2.rearrange("(k p) m -> p k m", p=128))
        cqS = sg.tile([128, KC], f32)
        nc.sync.dma_start(out=cqS, in_=d_cq)
        boS = sg.tile([128, KC], f32)
        nc.sync.dma_start(out=boS, in_=d_bo)
        b2S = sg.tile([128, KC], f32)
        nc.sync.dma_start(out=b2S, in_=d_b2)
        b1S = sg.tile([128, MC1], f32)
        nc.sync.dma_start(out=b1S, in_=d_b1)
        ohpS = sg.tile([128, 2], bf16)
        nc.sync.dma_start(out=ohpS, in_=d_ohp)
        onebS = sg.tile([128, 1], bf16)
        nc.vector.memset(onebS, 1.0)
        onerR = sg.tile([1, 128], f32r)
        on2R = sg.tile([2, 128], f32r)
        ckR = sg.tile([1, DIM], f32r)
        cvR = sg.tile([1, DIM], f32r)
        with tc.tile_pool(name="cstage", bufs=1) as cst:
            onerS = cst.tile([1, 128], f32)
            nc.vector.memset(onerS, 1.0)
            nc.vector.tensor_copy(out=onerR, in_=onerS)
            on2S = cst.tile([2, 128], f32)
            nc.sync.dma_start(out=on2S, in_=d_on2)
            nc.vector.tensor_copy(out=on2R, in_=on2S)
            ckS = cst.tile([1, DIM], f32)
            nc.sync.dma_start(out=ckS, in_=d_ck)
            nc.vector.tensor_copy(out=ckR, in_=ckS)
            cvS = cst.tile([1, DIM], f32)
            nc.sync.dma_start(out=cvS, in_=d_cv)
            nc.vector.tensor_copy(out=cvR, in_=cvS)
        epsS = sg.tile([1, 1], f32)
        nc.vector.memset(epsS, LN_EPS)
        epsqS = sg.tile([128, 1], f32)
        nc.vector.memset(epsqS, 1e-24)
        eps2S = sg.tile([2, 1], f32)
        nc.vector.memset(eps2S, 1e-24)

        # PSUM: psF 3 banks (ffn accumulators), psQ 2 banks (qproj/fold +
        # shared phase), psB 3 banks (stats + rank-1 broadcasts) = 8.
        # Separate pools keep the rings independent so the scheduler can
        # interleave block b+1's q-path matmuls with block b's FFN chains.
        psF = ctx.enter_context(tc.tile_pool(name="psF", bufs=3, space="PSUM"))
        psQ = ctx.enter_context(tc.tile_pool(name="psQ", bufs=2, space="PSUM"))
        psB = ctx.enter_context(tc.tile_pool(name="psB", bufs=3, space="PSUM"))

        # Row-stat helper: t3 = [128, KC, N] f32 SBUF tile.
        # Returns (mu_b, rstd_b) [128, N] f32 PSUM broadcast tiles.
        def row_stats(t3, N, tmp_pool, st_pool):
            s1 = psB.tile([1, N], f32, tag="st")
            s2 = psB.tile([1, N], f32, tag="st")
            for c in range(KC):
                xb = tmp_pool.tile([128, N], bf16, tag="xb")
                nc.vector.tensor_copy(out=xb, in_=t3[:, c, :])
                x2 = tmp_pool.tile([128, N], bf16, tag="x2")
                nc.scalar.activation(out=x2, in_=xb, func=AF.Square)
                nc.tensor.matmul(s1, lhsT=onebS, rhs=xb,
                                 start=(c == 0), stop=(c == KC - 1))
                nc.tensor.matmul(s2, lhsT=onebS, rhs=x2,
                                 start=(c == 0), stop=(c == KC - 1))
            mu = st_pool.tile([1, N], f32r, tag="mu")
            nc.vector.tensor_scalar_mul(out=mu, in0=s1, scalar1=1.0 / DIM)
            msq = st_pool.tile([1, N], f32, tag="stt")
            nc.vector.tensor_mul(out=msq, in0=mu.bitcast(f32),
                                 in1=mu.bitcast(f32))
            var = st_pool.tile([1, N], f32, tag="stt")
            nc.vector.scalar_tensor_tensor(out=var, in0=s2, scalar=1.0 / DIM,
                                           in1=msq, op0=MUL, op1=SUB)
            sq = st_pool.tile([1, N], f32, tag="stt")
            nc.scalar.activation(out=sq, in_=var, func=AF.Sqrt, bias=epsS)
            rstd_f = st_pool.tile([1, N], f32, tag="stt")
            nc.vector.reciprocal_approx_fast(out=rstd_f, in_=sq)
            rstd = st_pool.tile([1, N], f32r, tag="rstd")
            nc.vector.tensor_copy(out=rstd, in_=rstd_f)
            mu_b = psB.tile([128, N], f32, tag="st")
            nc.tensor.matmul(mu_b, lhsT=onerR, rhs=mu, start=True, stop=True)
            rstd_b = psB.tile([128, N], f32, tag="st")
            nc.tensor.matmul(rstd_b, lhsT=onerR, rhs=rstd,
                             start=True, stop=True)
            return mu_b, rstd_b

        # LN apply: xh3[:, c, :] = (t3[:, c, :] - mu_b) * rstd_b  (bf16 out)
        def ln_apply(t3, xh3, mu_b, rstd_b, N, tmp_pool):
            for c in range(KC):
                t1 = tmp_pool.tile([128, N], f32, tag="lnap")
                nc.vector.tensor_sub(out=t1, in0=t3[:, c, :], in1=mu_b)
                nc.vector.tensor_mul(out=xh3[:, c, :], in0=t1, in1=rstd_b)

        # ============ shared phase: prototypes -> W_tilde ============
        with tc.tile_pool(name="shp", bufs=1) as sp:
            phS = sp.tile([128, KC, NW], bf16)   # LN1-applied prototypes
            khS = sp.tile([128, KC, INNER], bf16)  # k_hat ROW-major
            vS = sp.tile([128, KC, INNER], bf16)   # v ROW-major

            with tc.tile_pool(name="shln", bufs=2) as sp2, \
                 tc.tile_pool(name="shsc", bufs=2) as spsc, \
                 tc.tile_pool(name="shst", bufs=2) as sps:
                for nb in range(4):              # four 256-col quarters of NW
                    NN = 256
                    cols = slice(nb * NN, (nb + 1) * NN)
                    pst = sps.tile([128, KC, NN], f32, tag="pst")
                    nc.sync.dma_start(out=pst, in_=pT3[:, :, cols])
                    mu_b, rstd_b = row_stats(pst, NN, sp2, spsc)
                    ln_apply(pst, phS[:, :, cols], mu_b, rstd_b, NN, sp2)

            # k/v projections, ROW-major: out[protos, inner] = ph.T @ w
            with tc.tile_pool(name="shpj", bufs=1) as spj, \
                 tc.tile_pool(name="shpt", bufs=3) as spt:
                for proj in ("k", "v"):
                    wS = spj.tile([128, KC, DIM], bf16, tag="wproj")
                    nc.sync.dma_start(
                        out=wS,
                        in_=(d_wk if proj == "k" else d_wv)
                        .rearrange("(k p) m -> p k m", p=128))
                    biasR = ckR if proj == "k" else cvR
                    for half in range(2):        # inner cols (8 heads each)
                        cs = slice(half * 512, (half + 1) * 512)
                        bias_b = psB.tile([128, 512], f32, tag="st")
                        nc.tensor.matmul(bias_b, lhsT=onerR,
                                         rhs=biasR[:, cs],
                                         start=True, stop=True)
                        bias_sb = spt.tile([128, 512], bf16, tag="bsb")
                        nc.scalar.activation(out=bias_sb, in_=bias_b,
                                             func=AF.Copy)
                        for c in range(KC):      # proto chunks
                            acc = psQ.tile([128, 512], f32, tag="mm")
                            for k in range(KC):
                                nc.tensor.matmul(
                                    acc,
                                    lhsT=phS[:, k, c * 128:(c + 1) * 128],
                                    rhs=wS[:, k, cs],
                                    start=(k == 0), stop=(k == KC - 1))
                            if proj == "v":
                                nc.vector.scalar_tensor_tensor(
                                    out=vS[:, c, cs], in0=acc, scalar=0.0,
                                    in1=bias_sb, op0=ADD, op1=ADD)
                            else:
                                kt = spt.tile([128, 512], bf16, tag="kt")
                                nc.vector.scalar_tensor_tensor(
                                    out=kt, in0=acc, scalar=0.0,
                                    in1=bias_sb, op0=ADD, op1=ADD)
                                k2 = spt.tile([128, 512], bf16, tag="k2")
                                nc.scalar.activation(out=k2, in_=kt,
                                                     func=AF.Square)
                                nrm2 = spt.tile([128, 8], f32, tag="nrm2")
                                nc.vector.reduce_sum(
                                    out=nrm2,
                                    in_=k2.rearrange("p (h d) -> p h d", d=DH),
                                    axis=mybir.AxisListType.X)
                                snc = spt.tile([128, 8], f32, tag="snc")
                                nc.scalar.activation(out=snc, in_=nrm2,
                                                     func=AF.Sqrt,
                                                     bias=epsqS[:, 0:1])
                                rn = spt.tile([128, 8], f32, tag="rn")
                                nc.vector.reciprocal_approx_fast(out=rn,
                                                                 in_=snc)
                                for h in range(8):
                                    nc.vector.tensor_scalar_mul(
                                        out=khS[:, c,
                                                half * 512 + h * DH:
                                                half * 512 + (h + 1) * DH],
                                        in0=kt[:, h * DH:(h + 1) * DH],
                                        scalar1=rn[:, h:h + 1])

            if DBG:
                nc.sync.dma_start(out=d_khdbg, in_=khS)
                nc.sync.dma_start(out=d_vdbg, in_=vS)

            # M^T per head (= v.T @ k_hat), then W_tilde = (M^T).T @ wo
            with tc.tile_pool(name="shm", bufs=1) as spm, \
                 tc.tile_pool(name="shwo", bufs=2) as swo:
                MTsb = spm.tile([64, INNER], bf16)
                for h in range(HEADS):
                    hs = slice(h * DH, (h + 1) * DH)
                    MT = psB.tile([64, DH], f32, tag="st")
                    for c in range(KC):
                        nc.tensor.matmul(MT, lhsT=vS[:, c, hs],
                                         rhs=khS[:, c, hs],
                                         start=(c == 0), stop=(c == KC - 1))
                    nc.scalar.activation(out=MTsb[:, hs], in_=MT, func=AF.Copy)
                for h in range(HEADS):
                    wo_h = swo.tile([64, DIM], bf16, tag="woh")
                    nc.sync.dma_start(out=wo_h,
                                      in_=d_wo[h * DH:(h + 1) * DH, :])
                    po = (h % 2) * 64
                    for half in range(2):
                        cs = slice(half * 512, (half + 1) * 512)
                        wt_h = psQ.tile([64, 512], f32, tag="mm")
                        nc.tensor.matmul(wt_h,
                                         lhsT=MTsb[:, h * DH:(h + 1) * DH],
                                         rhs=wo_h[:, cs],
                                         start=True, stop=True)
                        nc.scalar.activation(out=wtS[po:po + 64, h // 2, cs],
                                             in_=wt_h, func=AF.Copy)

        if DBG:
            nc.sync.dma_start(out=d_wtdbg, in_=wtS)

        # ============ main loop over query blocks ============
        mpA = ctx.enter_context(tc.tile_pool(name="mA", bufs=2))
        mpX = ctx.enter_context(tc.tile_pool(name="mX", bufs=1))
        mpG = ctx.enter_context(tc.tile_pool(name="mG", bufs=2))
        mpB = ctx.enter_context(tc.tile_pool(name="mB", bufs=3))
        mpSt = ctx.enter_context(tc.tile_pool(name="mSt", bufs=2))

        for blk in range(NBLK):
            cols = slice(blk * R, (blk + 1) * R)
            x = mpX.tile([128, KC, R], f32, tag="x")
            nc.sync.dma_start(out=x, in_=qT3[:, :, cols])
            mu_b, rstd_b = row_stats(x, R, mpB, mpSt)
            xh = mpA.tile([128, KC, R], bf16, tag="xh")
            ln_apply(x, xh, mu_b, rstd_b, R, mpB)

            # q projection; each 128-feature chunk holds exactly two
            # heads, so the L2 norm + scale is done per chunk (no
            # cross-chunk dependency -> fold matmuls start earlier).
            qh = mpA.tile([128, KC, R], bf16, tag="qh")
            for m in range(KC):
                zq = psQ.tile([128, R], f32, tag="mm")
                for k in range(KC):
                    nc.tensor.matmul(zq,
                                     lhsT=wqS[:, k, m * 128:(m + 1) * 128],
                                     rhs=xh[:, k, :],
                                     start=(k == 0), stop=(k == KC - 1))
                z2 = mpB.tile([128, R], bf16, tag="z2")
                nc.scalar.activation(out=z2, in_=zq, func=AF.Square,
                                     bias=cqS[:, m:m + 1])
                ssk = psB.tile([2, R], f32, tag="st")
                nc.tensor.matmul(ssk, lhsT=ohpS, rhs=z2,
                                 start=True, stop=True)
                snk = mpB.tile([2, R], f32, tag="snk")
                nc.scalar.activation(out=snk, in_=ssk, func=AF.Sqrt,
                                     bias=eps2S)
                snr = mpB.tile([2, R], f32, tag="snr")
                nc.vector.reciprocal_approx_fast(out=snr, in_=snk)
                snr_r = mpB.tile([2, R], f32r, tag="snrr")
                nc.vector.tensor_copy(out=snr_r, in_=snr)
                cb = psB.tile([128, R], f32, tag="st")
                nc.tensor.matmul(cb, lhsT=on2R, rhs=snr_r,
                                 start=True, stop=True)
                cbs = mpB.tile([128, R], bf16, tag="cbs")
                nc.scalar.activation(out=cbs, in_=cb, func=AF.Copy)
                nc.vector.scalar_tensor_tensor(
                    out=qh[:, m, :], in0=zq, scalar=cqS[:, m:m + 1],
                    in1=cbs, op0=ADD, op1=MUL)

            # attention+wo fold:  a = qh @ W_tilde + bo + x
            a = mpX.tile([128, KC, R], f32, tag="a")
            for m in range(KC):
                za = psQ.tile([128, R], f32, tag="mm")
                for k in range(KC):
                    nc.tensor.matmul(za,
                                     lhsT=wtS[:, k, m * 128:(m + 1) * 128],
                                     rhs=qh[:, k, :],
                                     start=(k == 0), stop=(k == KC - 1))
                nc.vector.scalar_tensor_tensor(
                    out=a[:, m, :], in0=za, scalar=boS[:, m:m + 1],
                    in1=x[:, m, :], op0=ADD, op1=ADD)

            if DBG:
                nc.sync.dma_start(out=d_adbg[:, :, cols], in_=a)
                nc.sync.dma_start(out=d_qhdbg[:, :, cols], in_=qh)

            mu2_b, rstd2_b = row_stats(a, R, mpB, mpSt)
            xh2 = mpA.tile([128, KC, R], bf16, tag="xh2")
            ln_apply(a, xh2, mu2_b, rstd2_b, R, mpB)

            # FFN1 + gelu (w1 is fp8 scaled by S1; descale via act scale)
            g = mpG.tile([128, MC1, R], bf16, tag="g")
            for m in range(MC1):
                zf = psF.tile([128, R], f32, tag="ffn")
                for k in range(KC):
                    nc.tensor.matmul(zf,
                                     lhsT=w1S[:, k, m * 128:(m + 1) * 128],
                                     rhs=xh2[:, k, :],
                                     start=(k == 0), stop=(k == KC - 1))
                nc.scalar.activation(out=g[:, m, :], in_=zf, func=AF.Gelu,
                                     bias=b1S[:, m:m + 1], scale=1.0 / S1)

            # FFN2 (w2 fp8 scaled by S2) + bias + residual, then store
            for m in range(KC):
                zy = psF.tile([128, R], f32, tag="ffn")
                for k in range(MC1):
                    nc.tensor.matmul(zy,
                                     lhsT=w2S[:, k, m * 128:(m + 1) * 128],
                                     rhs=g[:, k, :],
                                     start=(k == 0), stop=(k == MC1 - 1))
                yt = mpB.tile([128, R], f32, tag="yt")
                nc.vector.affine_then_add(out=yt, in0=zy, in1=a[:, m, :],
                                          scale=1.0 / S2,
                                          bias=b2S[:, m:m + 1])
                nc.sync.dma_start(out=out3[:, m, cols], in_=yt)

    nc.compile()
    return nc


def kernel(**inputs):
    import ml_dtypes
    from concourse.bass_utils import run_bass_kernel_spmd

    bf16 = ml_dtypes.bfloat16
    fp8 = ml_dtypes.float8_e4m3fn
    f32 = np.float32

    queries = np.asarray(inputs["queries"], dtype=f32)
    prototypes = np.asarray(inputs["prototypes"], dtype=f32)
    ln1_w = np.asarray(inputs["ln1_w"], dtype=f32)
    ln1_b = np.asarray(inputs["ln1_b"], dtype=f32)
    wq = np.asarray(inputs["wq"], dtype=f32)
    wk = np.asarray(inputs["wk"], dtype=f32)
    wv = np.asarray(inputs["wv"], dtype=f32)
    wo = np.asarray(inputs["wo"], dtype=f32)
    bo = np.asarray(inputs["bo"], dtype=f32)
    ln2_w = np.asarray(inputs["ln2_w"], dtype=f32)
    ln2_b = np.asarray(inputs["ln2_b"], dtype=f32)
    w1 = np.asarray(inputs["w1"], dtype=f32)
    b1 = np.asarray(inputs["b1"], dtype=f32)
    w2 = np.asarray(inputs["w2"], dtype=f32)
    b2 = np.asarray(inputs["b2"], dtype=f32)

    # ---- host-side folds (weights only) ----
    wq_f = (wq * ln1_w[:, None]).astype(bf16)      # [DIM, DIM]
    wk_f = (wk * ln1_w[:, None]).astype(bf16)
    wv_f = (wv * ln1_w[:, None]).astype(bf16)
    w1_8 = (w1 * ln2_w[:, None] * S1).astype(fp8)  # [DIM, MLP]
    w2_8 = (w2 * S2).astype(fp8)                   # [MLP, DIM]
    cq = (ln1_b @ wq).astype(f32)
    ck = (ln1_b @ wk).astype(f32)
    cv = (ln1_b @ wv).astype(f32)
    b1_f = (b1 + ln2_b @ w1).astype(f32)

    def cols128(v, nchunks):
        return np.ascontiguousarray(v.reshape(nchunks, 128).T).astype(f32)

    # per-half-chunk (one head = 64 partitions) ones masks
    ohp = np.zeros((128, 2), dtype=f32)
    ohp[:64, 0] = 1.0
    ohp[64:, 1] = 1.0
    on2 = np.ascontiguousarray(ohp.T)

    qT = np.ascontiguousarray(queries.T)           # [DIM, NQ]
    pT = np.ascontiguousarray(prototypes.T)        # [DIM, NW]

    common = {
        "pT": pT,
        "wq_f": wq_f, "wk_f": wk_f, "wv_f": wv_f,
        "wo_b": wo.astype(bf16),
        "w1_8": w1_8, "w2_8": w2_8,
        "cq_c": cols128(cq, KC),
        "ck_r": ck[None, :], "cv_r": cv[None, :],
        "bo_c": cols128(bo, KC),
        "b2_c": cols128(b2, KC), "b1_c": cols128(b1_f, MC1),
        "ohp": ohp.astype(bf16), "on2": on2,
    }
    in_maps = []
    for c in range(N_CORES):
        m = dict(common)
        m["qT"] = np.ascontiguousarray(qT[:, c * RPC:(c + 1) * RPC])
        in_maps.append(m)

    if "nc" not in _BUILD_CACHE:
        _BUILD_CACHE["nc"] = _build_nc()
    nc = _BUILD_CACHE["nc"]

    trace = bool(os.environ.get("KERNEL_TRACE"))
    res = run_bass_kernel_spmd(nc, in_maps, core_ids=list(range(N_CORES)),
                               trace=trace)
    _BUILD_CACHE["last_res"] = res
    yT = np.concatenate([res.results[c]["yT"] for c in range(N_CORES)], axis=1)
    return np.ascontiguousarray(yT.T)
